# revision 1
# baseline (speedup 1.0000x reference)
"""Masked multi-head attention kernel for 8 Trainium2 NeuronCores.

Strategy:
  - 24 (batch, head) pairs sharded as: core c -> batch c//4, heads [3*(c%4) .. 3*(c%4)+2].
  - Key-padding mask handled by HOST-side gather: only unmasked key positions are
    shipped/computed. Padded key slots get zeroed K columns (scores=0 -> exp=1)
    and a 0 in the indicator slot of V, so they contribute nothing.
  - Softmax without max-subtraction (scores ~ N(0,1); masked keys excluded).
  - Row-sum of exp folded into the AV matmul via an indicator slot on V:
    v_sb slots alternate [ind, v_h] so head h's AV lhsT is the contiguous
    slice [2h, 2h+1] (the PE weights AP must be a single free dim).
  - Attention processed in 512-query units (1 PSUM bank per tile) for deep
    software pipelining; 12 units = 3 heads x 4 query blocks.
  - Projection contraction packed: heads 0,1 share a K=128 matmul; head 2 K=64.
  - bf16 matmul inputs, fp32 PSUM accumulation, bf16 output partials
    (host sums the 4 partials per batch in fp32, adds proj_b).
  - DMA: issues spread across Sync+ACT queues; x shipped in paired C-chunks
    (8KB descriptors); output staged per query-block (6KB descriptors).
"""

import math

import numpy as np
import ml_dtypes

BF16 = ml_dtypes.bfloat16
B, N, C = 2, 2048, 768
H = 12
D = 64
HPC = 3          # heads per core
P = 128
QB = 512         # query block
NQB = N // QB
SCALE = D ** -0.5
NCORES = 8


def _build_program(KP: int):
    from concourse import bacc, mybir
    from concourse.tile import TileContext

    JG = KP // P
    f32 = mybir.dt.float32
    bf16 = mybir.dt.bfloat16
    nc = bacc.Bacc(None, target_bir_lowering=False)

    xT_d = nc.declare_dram_parameter("xT", [P, 6, N], bf16, False)
    xTk_d = nc.declare_dram_parameter("xTk", [P, 6, KP], bf16, False)
    kf_d = nc.declare_dram_parameter("kf", [P, JG, HPC, D], bf16, False)
    wq_d = nc.declare_dram_parameter("wqT", [P, 6, 192], bf16, False)
    wk_d = nc.declare_dram_parameter("wkT", [P, 6, 192], bf16, False)
    wv_d = nc.declare_dram_parameter("wvT", [P, 6, 192], bf16, False)
    pT01_d = nc.declare_dram_parameter("pT01", [P, 6, P], bf16, False)
    pT2_d = nc.declare_dram_parameter("pT2", [D, 6, P], bf16, False)
    out_d = nc.declare_dram_parameter("outT", [P, NQB, 6, QB], bf16, True)

    with TileContext(nc) as tc:
        with (
            tc.tile_pool(name="const", bufs=1) as cpool,
            tc.tile_pool(name="work", bufs=1) as wpool,
            tc.tile_pool(name="pt", bufs=4) as ptpool,
            tc.tile_pool(name="rb", bufs=4) as rbpool,
            tc.tile_pool(name="outp", bufs=2) as opool,
            tc.tile_pool(name="ps", bufs=2, space="PSUM") as pspool,
            tc.tile_pool(name="po", bufs=2, space="PSUM") as popool,
        ):
            # ---- input DMAs, spread across the two HWDGE issue queues
            # (Sync + ACT) so descriptor setup doesn't serialize; x shipped
            # with paired C-chunks for 8KB descriptors.
            wq = cpool.tile([P, 6, 192], bf16)
            xT = cpool.tile([P, 6, N], bf16)
            wk = cpool.tile([P, 6, 192], bf16)
            wv = cpool.tile([P, 6, 192], bf16)
            xTk = cpool.tile([P, 6, KP], bf16)
            kf = cpool.tile([P, JG, HPC, D], bf16)
            pT01 = cpool.tile([P, 6, P], bf16)
            pT2 = cpool.tile([D, 6, P], bf16)

            nc.sync.dma_start(wq[:], wq_d[:])
            nc.scalar.dma_start(xT[:, 0:1, :], xT_d[:, 0:1, :])
            nc.sync.dma_start(xT[:, 1:2, :], xT_d[:, 1:2, :])
            nc.sync.dma_start(xT[:, 2:3, :], xT_d[:, 2:3, :])
            nc.scalar.dma_start(xT[:, 3:4, :], xT_d[:, 3:4, :])
            nc.scalar.dma_start(xT[:, 4:5, :], xT_d[:, 4:5, :])
            nc.sync.dma_start(xT[:, 5:6, :], xT_d[:, 5:6, :])
            nc.scalar.dma_start(wk[:], wk_d[:])
            nc.sync.dma_start(wv[:], wv_d[:])
            nc.scalar.dma_start(xTk[:, 0:2, :], xTk_d[:, 0:2, :])
            nc.sync.dma_start(xTk[:, 2:4, :], xTk_d[:, 2:4, :])
            nc.scalar.dma_start(xTk[:, 4:6, :], xTk_d[:, 4:6, :])
            nc.sync.dma_start(kf[:], kf_d[:])
            nc.scalar.dma_start(pT01[:], pT01_d[:])
            nc.sync.dma_start(pT2[:], pT2_d[:])

            # ---- q/k projections: 2 heads packed per 128-row group, head 2
            # in a 64-row tile. PSUM chunks are 512 cols = 1 bank.
            qT01 = wpool.tile([P, N], bf16)
            qT2 = wpool.tile([D, N], bf16)
            kT01 = wpool.tile([P, KP], bf16)
            kT2 = wpool.tile([D, KP], bf16)

            for (w_sb, src, dst_list, ncols) in (
                (wq, xT, [(0, P, qT01), (P, D, qT2)], N),
                (wk, xTk, [(0, P, kT01), (P, D, kT2)], KP),
            ):
                for (m0, msz, dst) in dst_list:
                    for n0 in range(0, ncols, QB):
                        nsz = min(QB, ncols - n0)
                        pq = pspool.tile([P, QB], f32, name="pq", tag="ps")
                        for t in range(6):
                            nc.tensor.matmul(
                                pq[:msz, :nsz],
                                w_sb[:, t, m0 : m0 + msz],
                                src[:, t, n0 : n0 + nsz],
                                start=(t == 0),
                                stop=(t == 5),
                            )
                        nc.vector.tensor_copy(dst[:msz, n0 : n0 + nsz], pq[:msz, :nsz])

            # ---- V in [key, slot, dim] layout; even slots hold the keep
            # indicator, odd slots the per-head V rows, so head h's AV lhsT
            # is the contiguous slice slots [2h, 2h+1] (one free dim).
            v_sb = wpool.tile([P, JG, 2 * HPC, D], bf16)
            for jg in range(JG):
                nc.vector.tensor_copy(
                    v_sb[:, jg, 0 : 2 * HPC : 2, :], kf[:, jg, :, :]
                )
                pv = pspool.tile([P, QB], f32, tag="ps")
                for t in range(6):
                    nc.tensor.matmul(
                        pv[:, :192],
                        xTk[:, t, jg * P : (jg + 1) * P],
                        wv[:, t, :],
                        start=(t == 0),
                        stop=(t == 5),
                    )
                for ih in range(HPC):
                    nc.vector.tensor_copy(
                        v_sb[:, jg, 2 * ih + 1, :], pv[:, ih * D : (ih + 1) * D]
                    )

            # ---- attention in 512-query units + projection per query block.
            OT01 = wpool.tile([P, NQB, QB], bf16)
            OT2 = wpool.tile([D, NQB, QB], bf16)

            def att_unit(ih, qb):
                qsrc = qT01[D * ih : D * (ih + 1), :] if ih < 2 else qT2[:, :]
                ksrc = kT01[D * ih : D * (ih + 1), :] if ih < 2 else kT2[:, :]
                po = popool.tile([P, QB], f32, name="po", tag="po")
                qs = qsrc[:, qb * QB : (qb + 1) * QB]
                # key blocks processed in pairs: one exp instruction covers
                # both (the ACT engine is the attention bottleneck and each
                # exp carries ~260ns fixed overhead), and one exp completion
                # unblocks two AV matmuls. Each pair gets a fresh 2-slot PSUM
                # tile so the pool rotation keeps the pipeline 2-deep.
                npair = JG // 2
                if JG % 2:
                    # the odd block goes FIRST: its exp is half-latency, so
                    # the unit's first AV unblocks sooner at unit boundaries
                    jg = JG - 1
                    psc2 = pspool.tile([P, 2, QB], f32, name="psc2", tag="ps")
                    nc.tensor.matmul(
                        psc2[:, 0, :],
                        ksrc[:, jg * P : (jg + 1) * P],
                        qs,
                        start=True,
                        stop=True,
                    )
                    pt2 = ptpool.tile([P, 2, QB], bf16, name="pt2", tag="pt")
                    nc.scalar.activation(
                        pt2[:, 0, :],
                        psc2[:, 0, :],
                        mybir.ActivationFunctionType.Exp,
                        scale=float(SCALE),
                    )
                    nc.tensor.matmul(
                        po[:],
                        v_sb[:, jg, 2 * ih : 2 * ih + 2, :],
                        pt2[:, 0, :],
                        start=True,
                        stop=(JG == 1),
                    )
                for p in range(npair):
                    psc2 = pspool.tile([P, 2, QB], f32, name="psc2", tag="ps")
                    for s in range(2):
                        nc.tensor.matmul(
                            psc2[:, s, :],
                            ksrc[:, (2 * p + s) * P : (2 * p + s + 1) * P],
                            qs,
                            start=True,
                            stop=True,
                        )
                    pt2 = ptpool.tile([P, 2, QB], bf16, name="pt2", tag="pt")
                    nc.scalar.activation(
                        pt2[:],
                        psc2[:],
                        mybir.ActivationFunctionType.Exp,
                        scale=float(SCALE),
                    )
                    for s in range(2):
                        jg = 2 * p + s
                        nc.tensor.matmul(
                            po[:],
                            v_sb[:, jg, 2 * ih : 2 * ih + 2, :],
                            pt2[:, s, :],
                            start=(JG % 2 == 0 and jg == 0),
                            stop=(p == npair - 1 and s == 1),
                        )
                # partitions 0..D-1 of po hold the row-sum (indicator slot),
                # D..2D-1 hold O^T for this head.
                rb = rbpool.tile([D, QB], f32, tag="rb")
                nc.vector.reciprocal_approx_fast(rb[:], po[0:D, :])
                dst = OT01[D * ih : D * (ih + 1), qb, :] if ih < 2 else OT2[:, qb, :]
                nc.vector.tensor_mul(dst, po[D : 2 * D, :], rb[:])

            for qb in range(NQB):
                for ih in range(HPC):
                    att_unit(ih, qb)
                # projection for this query block (contraction packed 128+64)
                ob = opool.tile([P, 6, QB], bf16, tag="ob")
                for cg in range(6):
                    pp = pspool.tile([P, QB], f32, name="pp", tag="pp", bufs=2)
                    nc.tensor.matmul(
                        pp[:], pT01[:, cg, :], OT01[:, qb, :], start=True, stop=False
                    )
                    nc.tensor.matmul(
                        pp[:], pT2[:, cg, :], OT2[:, qb, :], start=False, stop=True
                    )
                    nc.vector.tensor_copy(ob[:, cg, :], pp[:])
                if qb % 2 == 0:
                    nc.sync.dma_start(out_d[:, qb, :, :], ob[:])
                else:
                    nc.scalar.dma_start(out_d[:, qb, :, :], ob[:])

    nc.finalize()
    return nc


def _prep_inputs(x, mask, qkv_w, proj_w):
    """Build the 8 per-core input maps. Returns (in_maps, KP)."""
    idx = [np.nonzero(mask[b] == 0.0)[0] for b in range(B)]
    nk = max(len(i) for i in idx)
    KP = max(P, int(math.ceil(nk / P)) * P)
    JG = KP // P

    per_batch = []
    for b in range(B):
        xTb = np.ascontiguousarray(x[b].T)  # [C, N] f32
        xT_in = xTb.reshape(6, P, N).transpose(1, 0, 2).astype(BF16)
        xk = np.zeros((C, KP), np.float32)
        xk[:, : len(idx[b])] = xTb[:, idx[b]]
        xTk_in = xk.reshape(6, P, KP).transpose(1, 0, 2).astype(BF16)
        kfv = np.zeros((KP,), np.float32)
        kfv[: len(idx[b])] = 1.0
        kf_in = np.ascontiguousarray(
            np.broadcast_to(
                kfv.reshape(JG, P).T[:, :, None, None], (P, JG, HPC, D)
            )
        ).astype(BF16)
        per_batch.append((xT_in, xTk_in, kf_in))

    in_maps = []
    for c in range(NCORES):
        b, g = c // 4, c % 4
        h0 = HPC * g
        xT_in, xTk_in, kf_in = per_batch[b]
        m = {"xT": xT_in, "xTk": xTk_in, "kf": kf_in}
        for name, off in (("wqT", 0), ("wkT", C), ("wvT", 2 * C)):
            w = qkv_w[off + h0 * D : off + (h0 + HPC) * D]  # [192, C]
            m[name] = (
                np.ascontiguousarray(w.T).reshape(6, P, 192).transpose(1, 0, 2).astype(BF16)
            )
        pw = proj_w[:, h0 * D : h0 * D + HPC * D]  # [768, 192]
        m["pT01"] = np.ascontiguousarray(pw[:, :P].T).reshape(P, 6, P).astype(BF16)
        m["pT2"] = np.ascontiguousarray(pw[:, P:].T).reshape(D, 6, P).astype(BF16)
        in_maps.append(m)
    return in_maps, KP


_CACHE = {}


def _get_program(KP):
    if KP not in _CACHE:
        _CACHE[KP] = _build_program(KP)
    return _CACHE[KP]


def _gather_output(results, proj_b):
    out = np.empty((B, N, C), np.float32)
    for b in range(B):
        acc = None
        for c in range(4 * b, 4 * b + 4):
            a = results[c]["outT"]  # [128, NQB, 6, QB] bf16
            a = np.asarray(a, np.float32).transpose(2, 0, 1, 3).reshape(C, N)
            acc = a if acc is None else acc + a
        out[b] = acc.T + proj_b[None, :]
    return out


def kernel(x, mask, qkv_w, proj_w, proj_b, _want_results=False):
    from concourse.bass_utils import run_bass_kernel_spmd

    x = np.asarray(x, np.float32)
    mask = np.asarray(mask, np.float32)
    qkv_w = np.asarray(qkv_w, np.float32)
    proj_w = np.asarray(proj_w, np.float32)
    proj_b = np.asarray(proj_b, np.float32)

    in_maps, KP = _prep_inputs(x, mask, qkv_w, proj_w)
    nc = _get_program(KP)
    res = run_bass_kernel_spmd(nc, in_maps, list(range(NCORES)))

    out = _gather_output(res.results, proj_b)
    if _want_results:
        return out, res
    return out

